# revision 1
# baseline (speedup 1.0000x reference)
"""JointNet (RNN-T) Bass kernel for trn2, 8 NeuronCores.

Math (per reference):
  he = enc @ W1[:D]           (B,T,H)
  hd = dec @ W1[D:]           (B,U,H)
  h  = gelu_tanh(he[:,:,None] + hd[:,None,:] + b1)    (B,T,U,H)
  out = h @ W2                (B,T,U,V)

Sharding: flatten (B,T) -> 1024 rows; core k takes rows [k*128,(k+1)*128)
(= batch b=k//2, t-range (k%2)*128..+128). W1/b1/W2 replicated.

The big matmul runs in fp8e4 with DoubleRow perf mode (2 k-tiles of 128
per instruction) using a 3-term split-precision scheme to stay within
the 2e-2 gate:
    out = h8@W8 + hl@W8 + h8@Wl          (hl@Wl dropped, ~1e-5 rel)
where h8 = e4m3(h), hl = e4m3(h - h8) on device, and W8 = e4m3(W2*SW),
Wl = e4m3(W2*SW - W8) pre-split on the host (SW=2048 keeps W2 out of the
fp8 subnormal range; the single common scale lets all three groups
accumulate into one PSUM tile, un-scaled at eviction).

Per-core device layout (H on partitions):
  heT[hc]  [128, T=128]  bf16 = We^T @ encT + b1    (4 H-chunks)
  hdT[hc]  [128, U=96]   bf16 = Wd^T @ decT
  x        [128, (t,u)]  bf16 = hdT bcast + heT bcast   (DVE)
  h        [128, (t,u)]  bf16 = gelu(x)                 (ACT)
  h8/hl    [128, 2, (t,u)] fp8 pair-plane tiles         (Pool/ACT cast, DVE sub)
  psum     [128 pairs, 1024] f32 = 12 DR matmuls (6 per V-half)
  evict    psum * (1/SW) -> bf16, split ACT/Pool/DVE; DMA per 128 rows
"""

import os
import numpy as np
import ml_dtypes

B, T, U, D, H, V = 4, 256, 96, 512, 512, 1024
NCORES = 8
TSH = (B * T) // NCORES          # 128 (b,t) rows per core
PAIRS = TSH * U                  # 12288 output rows per core
P = 128                          # partitions
DC = D // P                      # 4 contraction chunks for W1 matmuls
HC = H // P                      # 4 H chunks
SW = 2048.0                      # W2 fp8 scale (keeps W2*SW out of subnormals)

TRACE = False                    # test.py flips this to profile
LAST_RESULT = None               # BassKernelResults stash for test.py

_NC_CACHE = {}


def _build_module():
    import concourse.bass as bass
    import concourse.mybir as mybir
    import concourse.tile as tile
    from concourse import bacc

    f32 = mybir.dt.float32
    bf16 = mybir.dt.bfloat16
    fp8 = mybir.dt.float8e4
    DR = mybir.MatmulPerfMode.DoubleRow

    nc = bacc.Bacc("TRN2", target_bir_lowering=False, debug=False)
    encT = nc.dram_tensor("encT", [D, TSH], bf16, kind="ExternalInput")
    decT = nc.dram_tensor("decT", [D, U], bf16, kind="ExternalInput")
    w1 = nc.dram_tensor("W1", [2 * D, H], bf16, kind="ExternalInput")
    b1pc = nc.dram_tensor("b1pc", [P, HC], f32, kind="ExternalInput")
    # W2 hi/lo, interleaved for DoubleRow: [pair, p, ko*V + v]
    w8d = nc.dram_tensor("W8dr", [2, P, 2 * V], fp8, kind="ExternalInput")
    wld = nc.dram_tensor("Wldr", [2, P, 2 * V], fp8, kind="ExternalInput")
    out = nc.dram_tensor("out", [PAIRS, V], bf16, kind="ExternalOutput")

    with tile.TileContext(nc) as tc:
        with (
            tc.tile_pool(name="const", bufs=1) as const,
            tc.tile_pool(name="xbuf", bufs=3) as xbuf,
            tc.tile_pool(name="hbuf", bufs=3) as hbuf,
            tc.tile_pool(name="qbuf", bufs=3) as qbuf,
            tc.tile_pool(name="obuf", bufs=8) as obuf,
            tc.tile_pool(name="preps", bufs=2, space="PSUM") as preps,
            tc.tile_pool(name="mmps", bufs=3, space="PSUM") as mmps,
        ):
            # ---- PE warm-up: keep the PE busy during the load wait so
            # the p-state ramp (3us of continuous activity) completes
            # before real matmuls start ----
            warm_a = const.tile([P, P], bf16, tag="warma")
            nc.vector.memset(warm_a[:, :], 0.0)
            for _ in range(64):
                wps = preps.tile([P, TSH], f32, tag="pre", name="warm")
                nc.tensor.matmul(wps[:, :], warm_a[:, :], warm_a[:, :],
                                 start=True, stop=True)

            # ---- load params/acts: one consolidated DMA per tensor ----
            enc4 = const.tile([P, DC, TSH], bf16, tag="enc4")
            nc.sync.dma_start(
                out=enc4[:, :, :],
                in_=encT[:, :].rearrange("(dc p) t -> p dc t", p=P))
            we4 = const.tile([P, DC, H], bf16, tag="we4")
            nc.sync.dma_start(
                out=we4[:, :, :],
                in_=w1[0:D, :].rearrange("(dc p) h -> p dc h", p=P))
            b1_sb = const.tile([P, HC], f32, tag="b1")
            nc.sync.dma_start(out=b1_sb[:, :], in_=b1pc[:, :])
            dec4 = const.tile([P, DC, U], bf16, tag="dec4")
            nc.sync.dma_start(
                out=dec4[:, :, :],
                in_=decT[:, :].rearrange("(dc p) u -> p dc u", p=P))
            wd4 = const.tile([P, DC, H], bf16, tag="wd4")
            last_load = nc.sync.dma_start(
                out=wd4[:, :, :],
                in_=w1[D:2 * D, :].rearrange("(dc p) h -> p dc h", p=P))
            # ---- heT (+b1, bf16) and hdT (bf16) ----
            heT_sb, hdT_sb = [], []
            for hc in range(HC):
                ps = preps.tile([P, TSH], f32, tag="pre", name="pre_he")
                for dc in range(DC):
                    nc.tensor.matmul(
                        ps[:, :],
                        we4[:, dc, hc * P:(hc + 1) * P],
                        enc4[:, dc, :],
                        start=(dc == 0), stop=(dc == DC - 1),
                    )
                t_ = const.tile([P, TSH], bf16, tag=f"heT{hc}")
                nc.vector.tensor_scalar_add(t_[:, :], ps[:, :], b1_sb[:, hc:hc + 1])
                heT_sb.append(t_)
            for hc in range(HC):
                ps = preps.tile([P, U], f32, tag="pre", name="pre_hd")
                for dc in range(DC):
                    nc.tensor.matmul(
                        ps[:, :],
                        wd4[:, dc, hc * P:(hc + 1) * P],
                        dec4[:, dc, :],
                        start=(dc == 0), stop=(dc == DC - 1),
                    )
                t_ = const.tile([P, U], bf16, tag=f"hdT{hc}")
                nc.scalar.copy(t_[:, :], ps[:, :])
                hdT_sb.append(t_)

            # W2 hi/lo after the preamble loads (first DR is ~8us in; the
            # serialized HWDGE path must not delay the preamble loads).
            from concourse.tile_rust import add_dep_helper
            w8_sb, wl_sb = [], []
            for pair in range(2):
                t_ = const.tile([P, 2, V], fp8, tag=f"w8{pair}")
                d_ = nc.scalar.dma_start(out=t_[:, :, :], in_=w8d[pair, :, :])
                add_dep_helper(d_.ins, last_load.ins,
                               reason="defer W2 hi load behind preamble loads")
                w8_sb.append(t_)
            for pair in range(2):
                t_ = const.tile([P, 2, V], fp8, tag=f"wl{pair}")
                d_ = nc.scalar.dma_start(out=t_[:, :, :], in_=wld[pair, :, :])
                add_dep_helper(d_.ins, last_load.ins,
                               reason="defer W2 lo load behind preamble loads")
                wl_sb.append(t_)

            # ---- main loop over t-blocks (software-pipelined emission:
            # block i+1's add/gelu/cast/sub chain is emitted BEFORE block
            # i's DR matmuls + evictions so each engine's FIFO runs the
            # chain ahead of the heavy eviction work) ----
            gelu = mybir.ActivationFunctionType.Gelu_apprx_tanh
            TB = 16
            schedule = [4, 4, 4, 4] + [TB] * ((TSH - 16) // TB)
            inv_sw = 1.0 / SW
            ev = 0          # eviction round-robin counter

            def make_chain(t0c, tlen):
                R = tlen * U
                h_t = []
                for hc in range(HC):
                    x = xbuf.tile([P, R], bf16, tag=f"x{hc}", name=f"x{hc}")
                    x3 = x[:, :].rearrange("p (t u) -> p t u", u=U)
                    bc_hd = hdT_sb[hc][:, None, :].broadcast_to((P, tlen, U))
                    bc_he = heT_sb[hc][:, t0c:t0c + tlen, None].broadcast_to(
                        (P, tlen, U))
                    nc.vector.tensor_tensor(
                        out=x3, in0=bc_hd, in1=bc_he, op=mybir.AluOpType.add)
                    h = hbuf.tile([P, R], bf16, tag=f"h{hc}", name=f"h{hc}")
                    nc.scalar.activation(h[:, :], x[:, :], gelu)
                    h_t.append(h)
                h8p = [qbuf.tile([P, 2, R], fp8, tag="h8p0", name="h8p0"),
                       qbuf.tile([P, 2, R], fp8, tag="h8p1", name="h8p1")]
                hlp = [qbuf.tile([P, 2, R], fp8, tag="hlp0", name="hlp0"),
                       qbuf.tile([P, 2, R], fp8, tag="hlp1", name="hlp1")]
                for hc in range(HC):
                    pr, ko = hc >> 1, hc & 1
                    # cast h -> fp8 on Pool (GPSIMD can't see PSUM anyway)
                    nc.gpsimd.tensor_copy(h8p[pr][:, ko, :], h_t[hc][:, :])
                for hc in range(HC):
                    pr, ko = hc >> 1, hc & 1
                    # residual hl = h - h8: 3 on DVE, 1 on Pool
                    veng = nc.vector if hc < 3 else nc.gpsimd
                    veng.tensor_tensor(
                        out=hlp[pr][:, ko, :], in0=h_t[hc][:, :],
                        in1=h8p[pr][:, ko, :], op=mybir.AluOpType.subtract)
                return h8p, hlp

            def do_block(t0c, tlen, h8p, hlp):
                nonlocal ev
                R = tlen * U
                for blk in range(R // P):
                    c0 = blk * P
                    ps = mmps.tile([P, V], f32, tag="po", name="po")
                    groups = (
                        (h8p[0], w8_sb[0]), (h8p[1], w8_sb[1]),
                        (hlp[0], w8_sb[0]), (hlp[1], w8_sb[1]),
                        (h8p[0], wl_sb[0]), (h8p[1], wl_sb[1]),
                    )
                    # stationary-reuse order: same lhsT for both V-halves
                    for gi, (hq, wq) in enumerate(groups):
                        for vh in range(2):
                            nc.tensor.matmul(
                                ps[:, vh * (V // 2):(vh + 1) * (V // 2)],
                                hq[:, :, c0:c0 + P],
                                wq[:, :, vh * (V // 2):(vh + 1) * (V // 2)],
                                start=(gi == 0), stop=(gi == 5),
                                perf_mode=DR,
                            )
                    row0 = t0c * U + c0
                    ob = obuf.tile([P, V], bf16, tag="ob", name="ob")
                    if row0 + P >= PAIRS:
                        # final chunk: split eviction across both engines and
                        # DMA halves separately to shorten the drain tail
                        nc.scalar.mul(ob[:, 0:V // 2], ps[:, 0:V // 2], inv_sw)
                        nc.vector.tensor_scalar_mul(
                            ob[:, V // 2:V], ps[:, V // 2:V], inv_sw)
                        nc.sync.dma_start(
                            out=out[row0:row0 + P, 0:V // 2], in_=ob[:, 0:V // 2])
                        nc.sync.dma_start(
                            out=out[row0:row0 + P, V // 2:V], in_=ob[:, V // 2:V])
                    else:
                        # evictions: 3 ACT : 1 DVE, interleaved
                        if ev % 4 < 3:
                            nc.scalar.mul(ob[:, :], ps[:, :], inv_sw)
                        else:
                            nc.vector.tensor_scalar_mul(ob[:, :], ps[:, :], inv_sw)
                        ev += 1
                        nc.sync.dma_start(out=out[row0:row0 + P, :], in_=ob[:, :])

            t0c = 0
            for tlen in schedule:
                h8p, hlp = make_chain(t0c, tlen)
                do_block(t0c, tlen, h8p, hlp)
                t0c += tlen
    nc.compile()
    return nc


def _get_nc(mm_bf16=True):
    key = "g3"
    if key not in _NC_CACHE:
        _NC_CACHE[key] = _build_module()
    return _NC_CACHE[key]


def kernel(encoder_outputs, decoder_outputs, W1, b1, W2):
    global LAST_RESULT
    from concourse.bass_utils import run_bass_kernel_spmd

    bfl = ml_dtypes.bfloat16
    e4 = ml_dtypes.float8_e4m3
    enc = np.ascontiguousarray(np.asarray(encoder_outputs, dtype=np.float32).astype(bfl))
    dec = np.ascontiguousarray(np.asarray(decoder_outputs, dtype=np.float32).astype(bfl))
    w1 = np.ascontiguousarray(np.asarray(W1, dtype=np.float32).astype(bfl))
    b1v = np.asarray(b1, dtype=np.float32)
    w2 = np.asarray(W2, dtype=np.float32)

    # W2 hi/lo split at a single common scale, DoubleRow-interleaved:
    # [pair, p, ko, v] with contraction index = pair*256 + ko*128 + p.
    w8 = (w2 * SW).astype(e4)
    wl = (w2 * SW - w8.astype(np.float32)).astype(e4)
    w8dr = np.ascontiguousarray(
        w8.reshape(2, 2, P, V).transpose(0, 2, 1, 3).reshape(2, P, 2 * V))
    wldr = np.ascontiguousarray(
        wl.reshape(2, 2, P, V).transpose(0, 2, 1, 3).reshape(2, P, 2 * V))
    b1pc = np.ascontiguousarray(b1v.reshape(HC, P).T)   # [128, 4]

    nc = _get_nc()
    in_maps = []
    for k in range(NCORES):
        b = k // (T // TSH)
        t0 = (k % (T // TSH)) * TSH
        in_maps.append({
            "encT": np.ascontiguousarray(enc[b, t0:t0 + TSH, :].T),
            "decT": np.ascontiguousarray(dec[b].T),
            "W1": w1,
            "b1pc": b1pc,
            "W8dr": w8dr,
            "Wldr": wldr,
        })

    res = run_bass_kernel_spmd(
        nc, in_maps, core_ids=list(range(NCORES)), trace=TRACE)
    LAST_RESULT = res
    out = np.empty((B, T, U, V), dtype=np.float32)
    for k in range(NCORES):
        b = k // (T // TSH)
        t0 = (k % (T // TSH)) * TSH
        shard = res.results[k]["out"].reshape(TSH, U, V)
        out[b, t0:t0 + TSH] = shard.astype(np.float32)
    return out



# revision 2
# speedup vs baseline: 1.1600x; 1.1600x over previous
"""JointNet (RNN-T) Bass kernel for trn2, 8 NeuronCores.

Math (per reference):
  he = enc @ W1[:D]           (B,T,H)
  hd = dec @ W1[D:]           (B,U,H)
  h  = gelu_tanh(he[:,:,None] + hd[:,None,:] + b1)    (B,T,U,H)
  out = h @ W2                (B,T,U,V)

Sharding: flatten (B,T) -> 1024 rows; core k takes rows [k*128,(k+1)*128)
(= batch b=k//2, t-range (k%2)*128..+128). W1/b1/W2 replicated.

Precision: fp8e4 DoubleRow matmuls with a 2.5-term split:
    out = h8@W8 + h8@Wl + hl@W8[:256]
where h8 = e4m3(h), W8 = e4m3(W2*SW), Wl = e4m3(W2*SW - W8) (SW=2048 one
common scale; PSUM un-scaled at eviction), and hl = e4m3(h - h8) is the
h-quantization correction applied only to the 256 H-rows with the largest
quantization-error variance.  The H axis is permuted per core on the host
(applied to W1 columns / b1 / W2 rows) so those rows are chunks 0-1; the
permutation contracts away so the output needs no unpermute.  10 DR
matmuls per 128-row output block instead of the 12 a full 3-term needs.

Per-core device layout (H on partitions):
  heT   [128, 4, T=128] f32  = We^T @ encT + b1  (4 H-chunks)
  hdT   [128, 4, U=96]  bf16 = Wd^T @ decT
  x     [128, t, u]     bf16 = per-t DVE tensor_scalar add (4x perf mode):
                               x[:,t,:] = hdT[hc] + heT[hc][:,t]
  h8    chunks 2-3: ACT gelu writes fp8 directly
        chunks 0-1: ACT gelu -> bf16 h, Pool cast -> fp8, Pool sub -> hl
  psum  [128 rows, 1024] f32 = 10 DR matmuls (5 groups x 2 V-halves)
  evict psum * (1/SW) -> bf16, alternating ACT/DVE; DMA per 128 rows
"""

import numpy as np
import ml_dtypes

B, T, U, D, H, V = 4, 256, 96, 512, 512, 1024
NCORES = 8
TSH = (B * T) // NCORES          # 128 (b,t) rows per core
PAIRS = TSH * U                  # 12288 output rows per core
P = 128                          # partitions
DC = D // P                      # 4 contraction chunks for W1 matmuls
HC = H // P                      # 4 H chunks
SW = 2048.0                      # W2 fp8 scale (keeps W2*SW out of subnormals)

TRACE = False                    # test.py flips this to profile
LAST_RESULT = None               # BassKernelResults stash for test.py

_NC_CACHE = {}


def _build_module():
    import concourse.bass as bass
    import concourse.mybir as mybir
    import concourse.tile as tile
    from concourse import bacc

    f32 = mybir.dt.float32
    bf16 = mybir.dt.bfloat16
    fp8 = mybir.dt.float8e4
    DR = mybir.MatmulPerfMode.DoubleRow

    nc = bacc.Bacc("TRN2", target_bir_lowering=False, debug=False)
    encT = nc.dram_tensor("encT", [D, TSH], bf16, kind="ExternalInput")
    decT = nc.dram_tensor("decT", [D, U], bf16, kind="ExternalInput")
    w1 = nc.dram_tensor("W1", [2 * D, H], bf16, kind="ExternalInput")
    b1pc = nc.dram_tensor("b1pc", [P, HC], f32, kind="ExternalInput")
    # W2 hi/lo, interleaved for DoubleRow: [pair, p, ko*V + v]
    w8d = nc.dram_tensor("W8dr", [2, P, 2 * V], fp8, kind="ExternalInput")
    wld = nc.dram_tensor("Wldr", [2, P, 2 * V], fp8, kind="ExternalInput")
    out = nc.dram_tensor("out", [PAIRS, V], bf16, kind="ExternalOutput")

    with tile.TileContext(nc) as tc:
        with (
            tc.tile_pool(name="const", bufs=1) as const,
            tc.tile_pool(name="xbuf", bufs=3) as xbuf,
            tc.tile_pool(name="hbuf", bufs=3) as hbuf,
            tc.tile_pool(name="qbuf", bufs=3) as qbuf,
            tc.tile_pool(name="obuf", bufs=8) as obuf,
            tc.tile_pool(name="preps", bufs=2, space="PSUM") as preps,
            tc.tile_pool(name="mmps", bufs=3, space="PSUM") as mmps,
        ):
            # ---- PE warm-up: keep the PE busy during the load wait so
            # the p-state ramp (3us of continuous activity) completes
            # before real matmuls start ----
            warm_a = const.tile([P, P], bf16, tag="warma")
            nc.vector.memset(warm_a[:, :], 0.0)
            for _ in range(64):
                wps = preps.tile([P, TSH], f32, tag="pre", name="warm")
                nc.tensor.matmul(wps[:, :], warm_a[:, :], warm_a[:, :],
                                 start=True, stop=True)

            # ---- load params/acts: one consolidated DMA per tensor ----
            enc4 = const.tile([P, DC, TSH], bf16, tag="enc4")
            nc.sync.dma_start(
                out=enc4[:, :, :],
                in_=encT[:, :].rearrange("(dc p) t -> p dc t", p=P))
            we4 = const.tile([P, DC, H], bf16, tag="we4")
            nc.sync.dma_start(
                out=we4[:, :, :],
                in_=w1[0:D, :].rearrange("(dc p) h -> p dc h", p=P))
            b1_sb = const.tile([P, HC], f32, tag="b1")
            nc.sync.dma_start(out=b1_sb[:, :], in_=b1pc[:, :])
            dec4 = const.tile([P, DC, U], bf16, tag="dec4")
            nc.sync.dma_start(
                out=dec4[:, :, :],
                in_=decT[:, :].rearrange("(dc p) u -> p dc u", p=P))
            wd4 = const.tile([P, DC, H], bf16, tag="wd4")
            last_load = nc.sync.dma_start(
                out=wd4[:, :, :],
                in_=w1[D:2 * D, :].rearrange("(dc p) h -> p dc h", p=P))
            # ---- heT (f32, +b1) and hdT (bf16) ----
            heT = const.tile([P, HC, TSH], f32, tag="heT")
            hdT = const.tile([P, HC, U], bf16, tag="hdT")
            for hc in range(HC):
                ps = preps.tile([P, TSH], f32, tag="pre", name="pre_he")
                for dc in range(DC):
                    nc.tensor.matmul(
                        ps[:, :],
                        we4[:, dc, hc * P:(hc + 1) * P],
                        enc4[:, dc, :],
                        start=(dc == 0), stop=(dc == DC - 1),
                    )
                nc.vector.tensor_scalar_add(heT[:, hc, :], ps[:, :],
                                            b1_sb[:, hc:hc + 1])
            for hc in range(HC):
                ps = preps.tile([P, U], f32, tag="pre", name="pre_hd")
                for dc in range(DC):
                    nc.tensor.matmul(
                        ps[:, :],
                        wd4[:, dc, hc * P:(hc + 1) * P],
                        dec4[:, dc, :],
                        start=(dc == 0), stop=(dc == DC - 1),
                    )
                nc.scalar.copy(hdT[:, hc, :], ps[:, :])

            # W2 hi/lo after the preamble loads (first DR is ~8us in; the
            # serialized HWDGE path must not delay the preamble loads).
            from concourse.tile_rust import add_dep_helper
            w8_sb, wl_sb = [], []
            for pair in range(2):
                t_ = const.tile([P, 2, V], fp8, tag=f"w8{pair}")
                d_ = nc.scalar.dma_start(out=t_[:, :, :], in_=w8d[pair, :, :])
                add_dep_helper(d_.ins, last_load.ins,
                               reason="defer W2 hi load behind preamble loads")
                w8_sb.append(t_)
            for pair in range(2):
                t_ = const.tile([P, 2, V], fp8, tag=f"wl{pair}")
                d_ = nc.scalar.dma_start(out=t_[:, :, :], in_=wld[pair, :, :])
                add_dep_helper(d_.ins, last_load.ins,
                               reason="defer W2 lo load behind preamble loads")
                wl_sb.append(t_)

            # ---- main loop over t-blocks ----
            gelu = mybir.ActivationFunctionType.Gelu_apprx_tanh
            TB = 16
            schedule = [4, 4, 4, 4] + [TB] * ((TSH - 16) // TB)
            inv_sw = 1.0 / SW
            ev = 0          # eviction round-robin counter

            def make_chain(t0c, tlen):
                R = tlen * U
                h8p = [qbuf.tile([P, 2, R], fp8, tag="h8p0", name="h8p0"),
                       qbuf.tile([P, 2, R], fp8, tag="h8p1", name="h8p1")]
                hl2p = qbuf.tile([P, 2, R], fp8, tag="hl2p", name="hl2p")
                # uncorrected half (chunks 2,3): x-add then gelu -> fp8
                for hc in (2, 3):
                    x = xbuf.tile([P, tlen, U], bf16, tag=f"x{hc}",
                                  name=f"x{hc}")
                    for tt in range(tlen):
                        nc.vector.tensor_scalar_add(
                            x[:, tt, :], hdT[:, hc, :],
                            heT[:, hc, t0c + tt:t0c + tt + 1])
                    nc.scalar.activation(
                        h8p[1][:, hc - 2, :],
                        x[:, :, :].rearrange("p t u -> p (t u)"), gelu)
                # corrected half (chunks 0,1): gelu -> bf16 h, cast, sub
                h_t = {}
                for hc in (0, 1):
                    x = xbuf.tile([P, tlen, U], bf16, tag=f"x{hc}",
                                  name=f"x{hc}")
                    for tt in range(tlen):
                        nc.vector.tensor_scalar_add(
                            x[:, tt, :], hdT[:, hc, :],
                            heT[:, hc, t0c + tt:t0c + tt + 1])
                    h = hbuf.tile([P, R], bf16, tag=f"h{hc}", name=f"h{hc}")
                    nc.scalar.activation(
                        h[:, :], x[:, :, :].rearrange("p t u -> p (t u)"),
                        gelu)
                    h_t[hc] = h
                for hc in (0, 1):
                    nc.gpsimd.tensor_copy(h8p[0][:, hc, :], h_t[hc][:, :])
                for hc in (0, 1):
                    nc.gpsimd.tensor_tensor(
                        out=hl2p[:, hc, :], in0=h_t[hc][:, :],
                        in1=h8p[0][:, hc, :], op=mybir.AluOpType.subtract)
                return h8p, hl2p

            def do_block(t0c, tlen, h8p, hl2p):
                nonlocal ev
                R = tlen * U
                for blk in range(R // P):
                    c0 = blk * P
                    ps = mmps.tile([P, V], f32, tag="po", name="po")
                    groups = (
                        (h8p[0], w8_sb[0]), (h8p[1], w8_sb[1]),
                        (h8p[0], wl_sb[0]), (h8p[1], wl_sb[1]),
                        (hl2p, w8_sb[0]),
                    )
                    # stationary-reuse order: same lhsT for both V-halves
                    for gi, (hq, wq) in enumerate(groups):
                        for vh in range(2):
                            nc.tensor.matmul(
                                ps[:, vh * (V // 2):(vh + 1) * (V // 2)],
                                hq[:, :, c0:c0 + P],
                                wq[:, :, vh * (V // 2):(vh + 1) * (V // 2)],
                                start=(gi == 0), stop=(gi == 4),
                                perf_mode=DR,
                            )
                    row0 = t0c * U + c0
                    ob = obuf.tile([P, V], bf16, tag="ob", name="ob")
                    if row0 + P >= PAIRS:
                        # final chunk: split eviction across both engines and
                        # DMA halves separately to shorten the drain tail
                        nc.scalar.mul(ob[:, 0:V // 2], ps[:, 0:V // 2], inv_sw)
                        nc.vector.tensor_scalar_mul(
                            ob[:, V // 2:V], ps[:, V // 2:V], inv_sw)
                        nc.sync.dma_start(
                            out=out[row0:row0 + P, 0:V // 2], in_=ob[:, 0:V // 2])
                        nc.sync.dma_start(
                            out=out[row0:row0 + P, V // 2:V], in_=ob[:, V // 2:V])
                    else:
                        # evictions alternate ACT / DVE
                        if ev % 2 == 0:
                            nc.scalar.mul(ob[:, :], ps[:, :], inv_sw)
                        else:
                            nc.vector.tensor_scalar_mul(ob[:, :], ps[:, :], inv_sw)
                        ev += 1
                        nc.sync.dma_start(out=out[row0:row0 + P, :], in_=ob[:, :])

            t0c = 0
            for tlen in schedule:
                h8p, hl2p = make_chain(t0c, tlen)
                do_block(t0c, tlen, h8p, hl2p)
                t0c += tlen
    nc.compile()
    return nc


def _get_nc(mm_bf16=True):
    key = "g4"
    if key not in _NC_CACHE:
        _NC_CACHE[key] = _build_module()
    return _NC_CACHE[key]


def _gelu_tanh(x):
    return 0.5 * x * (1.0 + np.tanh(np.sqrt(2 / np.pi) * (x + 0.044715 * x ** 3)))


def _rank_order(enc_b, dec_b, W1b, b1v):
    """Per-core H permutation: H-rows ranked by h fp8-quantization-error
    variance (sampled over the (t,u) grid), largest first."""
    bfl = ml_dtypes.bfloat16
    e4 = ml_dtypes.float8_e4m3
    he = (enc_b[::4].astype(np.float32) @ W1b[:D].astype(np.float32)) + b1v
    hd = dec_b.astype(np.float32) @ W1b[D:].astype(np.float32)
    x = (he[:, None, :] + hd[None, :, :]).astype(bfl).astype(np.float32)
    h = _gelu_tanh(x).astype(bfl).astype(np.float32)
    d = h - h.astype(e4).astype(np.float32)
    var_k = (d.reshape(-1, H) ** 2).sum(axis=0)
    return np.argsort(-var_k)


def kernel(encoder_outputs, decoder_outputs, W1, b1, W2):
    global LAST_RESULT
    from concourse.bass_utils import run_bass_kernel_spmd

    bfl = ml_dtypes.bfloat16
    e4 = ml_dtypes.float8_e4m3
    enc = np.ascontiguousarray(np.asarray(encoder_outputs, dtype=np.float32).astype(bfl))
    dec = np.ascontiguousarray(np.asarray(decoder_outputs, dtype=np.float32).astype(bfl))
    w1 = np.ascontiguousarray(np.asarray(W1, dtype=np.float32).astype(bfl))
    b1v = np.asarray(b1, dtype=np.float32)
    w2 = np.asarray(W2, dtype=np.float32)

    nc = _get_nc()
    in_maps = []
    for k in range(NCORES):
        b = k // (T // TSH)
        t0 = (k % (T // TSH)) * TSH
        order = _rank_order(enc[b, t0:t0 + TSH], dec[b], w1, b1v)
        w1p = np.ascontiguousarray(w1[:, order])
        b1p = b1v[order]
        w2p = w2[order]
        # W2 hi/lo split at a single common scale, DoubleRow-interleaved:
        # [pair, p, ko, v] with contraction index = pair*256 + ko*128 + p.
        w8 = (w2p * SW).astype(e4)
        wl = (w2p * SW - w8.astype(np.float32)).astype(e4)
        w8dr = np.ascontiguousarray(
            w8.reshape(2, 2, P, V).transpose(0, 2, 1, 3).reshape(2, P, 2 * V))
        wldr = np.ascontiguousarray(
            wl.reshape(2, 2, P, V).transpose(0, 2, 1, 3).reshape(2, P, 2 * V))
        b1pc = np.ascontiguousarray(b1p.reshape(HC, P).T)   # [128, 4]
        in_maps.append({
            "encT": np.ascontiguousarray(enc[b, t0:t0 + TSH, :].T),
            "decT": np.ascontiguousarray(dec[b].T),
            "W1": w1p,
            "b1pc": b1pc,
            "W8dr": w8dr,
            "Wldr": wldr,
        })

    res = run_bass_kernel_spmd(
        nc, in_maps, core_ids=list(range(NCORES)), trace=TRACE)
    LAST_RESULT = res
    out = np.empty((B, T, U, V), dtype=np.float32)
    for k in range(NCORES):
        b = k // (T // TSH)
        t0 = (k % (T // TSH)) * TSH
        shard = res.results[k]["out"].reshape(TSH, U, V)
        out[b, t0:t0 + TSH] = shard.astype(np.float32)
    return out
